# revision 1
# baseline (speedup 1.0000x reference)
"""CrystalGraphConvNet on 8 trn2 NeuronCores (Bass/Tile), self-contained.

v2: single indirect gather per tile ([L,12] offset AP), 3-neighbor-packed
q-matmuls via host block-diag weights, full-width vector ops with
broadcast-AP self-term add, native Sigmoid/Softplus activations.
"""
import os

import numpy as np
import ml_dtypes

import concourse.bass as bass
import concourse.mybir as mybir
import concourse.tile as tile
from concourse.bass import IndirectOffsetOnAxis
from concourse.bass_utils import run_bass_kernel_spmd
from concourse.masks import make_identity

F32 = mybir.dt.float32
BF16 = mybir.dt.bfloat16
I32 = mybir.dt.int32
FP8 = mybir.dt.float8e4
AF = mybir.ActivationFunctionType
ALU = mybir.AluOpType

N, M, ORIG, NBR, F, H, N0 = 100000, 12, 92, 41, 64, 128, 2000
NC = 8
S = N // NC
EPS = 1e-5
TILES = [(t * 128, min(128, S - t * 128)) for t in range((S + 127) // 128)]
NT = len(TILES)
CLOC = 512
CB = N0 // NC  # 250
BASES = [max(0, k * CB - 128) for k in range(NC)]
N0P = 2560
K3 = 3 * NBR  # 123
G4 = 4        # column groups of 3 neighbors each
GW = 3 * 2 * F  # 384

_ctr = [0]


def split_sync_waits(nc, limit=1):
    f = nc.m.functions[0]
    for b in f.blocks:
        new, changed = [], False
        for i in b.instructions:
            si = i.sync_info
            if si is not None and len(si.on_wait) > limit:
                waits = list(si.on_wait)
                head, rest = waits[:-limit], waits[-limit:]
                for k in range(0, len(head), limit):
                    _ctr[0] += 1
                    d = mybir.InstNoOp(name=f"waitsplit-{_ctr[0]}", ins=[], outs=[])
                    d.engine = i.engine
                    d.sync_info = mybir.SyncInfo(on_wait=head[k:k + limit], on_update=[])
                    new.append(d)
                si.on_wait = rest
                changed = True
            new.append(i)
        if changed:
            b.instructions = new


def build_program():
    nc = bass.Bass(num_devices=NC, dynamic_dma_scratch_size=24576,
                   num_swdge_queues=4)

    atomT = nc.dram_tensor("atomT", [ORIG + 1, S], F32, kind="ExternalInput")
    nbrT = nc.dram_tensor("nbrT", [NT, K3, 512], BF16, kind="ExternalInput")
    idxp = nc.dram_tensor("idxp", [128, NT * M], I32, kind="ExternalInput")
    cidp = nc.dram_tensor("cidp", [128, NT], I32, kind="ExternalInput")
    invc = nc.dram_tensor("invc", [1, N0P], F32, kind="ExternalInput")
    wemb = nc.dram_tensor("wemb", [ORIG + 1, F], F32, kind="ExternalInput")
    wsp = nc.dram_tensor("wsp", [2, F + 1, 4 * F], F32, kind="ExternalInput")
    wnbrf3 = nc.dram_tensor("wnbrf3", [2, K3, GW], BF16, kind="ExternalInput")
    bn1g = nc.dram_tensor("bn1g", [2, 1, 2 * F], F32, kind="ExternalInput")
    bn1b = nc.dram_tensor("bn1b", [2, 1, 2 * F], F32, kind="ExternalInput")
    bn2g = nc.dram_tensor("bn2g", [2, 1, F], F32, kind="ExternalInput")
    bn2b = nc.dram_tensor("bn2b", [2, 1, F], F32, kind="ExternalInput")
    wfc = nc.dram_tensor("wfc", [F + 1, H], F32, kind="ExternalInput")
    wout = nc.dram_tensor("wout", [H, 1], F32, kind="ExternalInput")
    bout = nc.dram_tensor("bout", [1, 1], F32, kind="ExternalInput")

    out = nc.dram_tensor("out", [1, N0], F32, kind="ExternalOutput")

    ag_in = nc.dram_tensor("ag_in", [S, 2 * F], FP8)
    a_full = nc.dram_tensor("a_full", [N, 2 * F], FP8, addr_space="Shared")
    stash = nc.dram_tensor("stash", [NT, 128, M * 2 * F], BF16)
    cc_in = nc.dram_tensor("cc_in", [1, 1024], F32)
    cc_out = nc.dram_tensor("cc_out", [1, 1024], F32, addr_space="Shared")
    cc2_in = nc.dram_tensor("cc2_in", [1, 2 * F], F32)
    cc2_out = nc.dram_tensor("cc2_out", [1, 2 * F], F32, addr_space="Shared")
    cr_in = nc.dram_tensor("cr_in", [F, CLOC], F32)
    cr_out = nc.dram_tensor("cr_out", [NC, F, CLOC], F32, addr_space="Shared")

    RG = [list(range(NC))]

    with tile.TileContext(nc) as tc:
        with tc.tile_pool(name="persist", bufs=1) as pp, \
             tc.tile_pool(name="work", bufs=3) as wp, \
             tc.tile_pool(name="small", bufs=1) as sp:

            ident_b = pp.tile([128, 128], BF16)
            make_identity(nc, ident_b[:])
            ones_col = pp.tile([128, 1], F32)
            nc.vector.memset(ones_col[:], 1.0)
            ones_col_b = pp.tile([128, 1], BF16)
            nc.vector.memset(ones_col_b[:], 1.0)
            ones_row = pp.tile([1, 128], F32)
            nc.vector.memset(ones_row[:], 1.0)

            a_sb = pp.tile([128, NT * F], BF16, tag="a_sb")
            a1_sb = pp.tile([128, NT * F], BF16, tag="a1_sb")
            sum_sb = pp.tile([128, NT * F], BF16, tag="sum_sb")
            s_sb = pp.tile([128, NT * 2 * F], BF16, tag="s_sb")
            scb12 = pp.tile([128, M * 2 * F], BF16, tag="scb12")
            bib12 = pp.tile([128, M * 2 * F], BF16, tag="bib12")
            idx_sb = pp.tile([128, NT * M], I32, tag="idx_sb")
            nc.sync.dma_start(idx_sb[:], idxp[:])

            def softplus_sep(out, x):
                """out = softplus(x) = ln(1+e^x); |x| stays < ~15 here so the
                direct form is safe (no |x| splitting needed)."""
                nc.scalar.activation(out, x, AF.Exp)
                nc.scalar.activation(out, out, AF.Ln, bias=1.0)

            def rsqrt_row(dst, src, width, tmp_pool):
                """dst = 1/sqrt(src) elementwise on a [1, width] f32 row (DVE only)."""
                yi = tmp_pool.tile([1, width], I32, tag="rsq_i")
                nc.vector.tensor_scalar(
                    out=yi[:], in0=src.bitcast(I32), scalar1=1, scalar2=None,
                    op0=ALU.logical_shift_right)
                nc.vector.tensor_scalar(
                    out=yi[:], in0=yi[:], scalar1=-1, scalar2=0x5f3759df,
                    op0=ALU.mult, op1=ALU.add)
                y = yi[:].bitcast(F32)
                t = tmp_pool.tile([1, width], F32, tag="rsq_t")
                for _ in range(4):
                    nc.vector.tensor_mul(t[:], y, y)
                    nc.vector.tensor_mul(t[:], t[:], src)
                    nc.vector.tensor_scalar(
                        out=t[:], in0=t[:], scalar1=-0.5, scalar2=1.5,
                        op0=ALU.mult, op1=ALU.add)
                    nc.vector.tensor_mul(y, y, t[:])
                nc.vector.tensor_copy(dst, y)

            # ---- embedding ----
            wemb_sb = sp.tile([ORIG + 1, F], F32, tag="emb_w")
            nc.sync.dma_start(wemb_sb[:], wemb[:])
            with tc.tile_pool(name="ps_emb", bufs=2, space="PSUM") as pse:
                for t, (o, L) in enumerate(TILES):
                    lhs = wp.tile([ORIG + 1, 128], F32, tag="emb_lhs")
                    nc.sync.dma_start(lhs[:, :L], atomT[:, o:o + L])
                    ps = pse.tile([128, F], F32, tag="emb_ps")
                    nc.tensor.matmul(ps[:L, :], lhs[:, :L], wemb_sb[:],
                                     start=True, stop=True)
                    nc.vector.tensor_copy(a_sb[:L, t * F:(t + 1) * F], ps[:L, :])

            def conv_layer(l, ain_sb, aout_sb, extra_res):
                # ---- phase 0: abT, [s | p] = a1 @ [Wself+b | Wnbr], AllGather ----
                wsl = sp.tile([F + 1, 4 * F], BF16, tag="wsl")
                wslf = sp.tile([F + 1, 4 * F], F32, tag="wslf")
                nc.sync.dma_start(wslf[:], wsp[l])
                nc.vector.tensor_copy(wsl[:], wslf[:])
                with tc.tile_pool(name=f"ps0_{l}", bufs=2, space="PSUM") as ps0:
                    for t, (o, L) in enumerate(TILES):
                        at = ain_sb[:L, t * F:(t + 1) * F]
                        ab = wp.tile([128, F], BF16, tag="ph0ab")
                        nc.vector.tensor_copy(ab[:L, :], at)
                        ps = ps0.tile([128, 128], BF16, tag="tps")
                        nc.tensor.transpose(ps[:F, :L], ab[:L, :], ident_b[:L, :L])
                        abT = wp.tile([F + 1, 128], BF16, tag="abT")
                        nc.vector.tensor_copy(abT[:F, :L], ps[:F, :L])
                        nc.vector.memset(abT[F:F + 1, :], 1.0)
                        ps2 = ps0.tile([128, 4 * F], F32, tag="sps")
                        nc.tensor.matmul(ps2[:L, :], abT[:, :L],
                                         wsl[:], start=True, stop=True)
                        nc.vector.tensor_copy(s_sb[:L, t * 2 * F:(t + 1) * 2 * F],
                                              ps2[:L, :2 * F])
                        pb = wp.tile([128, 2 * F], FP8, tag="pb")
                        nc.vector.tensor_copy(pb[:L, :], ps2[:L, 2 * F:])
                        nc.sync.dma_start(ag_in[o:o + L, :], pb[:L, :])

                nc.gpsimd.collective_compute(
                    "AllGather", ALU.bypass, replica_groups=RG,
                    ins=[ag_in[:]], outs=[a_full[:]])

                wnf3 = sp.tile([K3, GW], BF16, tag="wnf3")
                nc.sync.dma_start(wnf3[:], wnbrf3[l])

                # ---- pass A ----
                with tc.tile_pool(name=f"psA_{l}", bufs=2, space="PSUM") as psA, \
                     tc.tile_pool(name=f"psS_{l}", bufs=1, space="PSUM") as psS:
                    st_ps = psS.tile([1, 512], F32, tag="st_x")
                    st2_ps = psS.tile([1, 512], F32, tag="st_x2")
                    for t, (o, L) in enumerate(TILES):
                        nt = wp.tile([K3, 512], BF16, tag="nbrt")
                        nc.sync.dma_start(nt[:], nbrT[t])
                        g0 = wp.tile([128, GW], FP8, tag="g0")
                        g1 = wp.tile([128, GW], FP8, tag="g1")
                        g2t = wp.tile([128, GW], FP8, tag="g1b")
                        g3 = wp.tile([128, GW], FP8, tag="g3")
                        gs = [g0, g1, g2t, g3]
                        # round-robin across 4 dst tiles: consecutive gathers
                        # hit different tiles, breaking per-tile WAW chains
                        for j in range(3):
                            for gq in range(G4):
                                m = gq * 3 + j
                                gi = nc.gpsimd.indirect_dma_start(
                                    out=gs[gq][:L, j * 2 * F:(j + 1) * 2 * F],
                                    out_offset=None, in_=a_full[:],
                                    in_offset=IndirectOffsetOnAxis(
                                        ap=idx_sb[:L, t * M + m:t * M + m + 1],
                                        axis=0))
                                if m % 4:
                                    gi.queue = f"qPoolDynamic{m % 4}"

                        gated = wp.tile([128, M * 2 * F], BF16, tag="gated")
                        for gq in range(G4):
                            pq = psA.tile([128, GW], F32, tag="pqps")
                            nc.tensor.matmul(pq[:L, :],
                                             nt[:, gq * 128:gq * 128 + L],
                                             wnf3[:], start=True, stop=True)
                            gsl = slice(gq * GW, (gq + 1) * GW)
                            nc.vector.tensor_tensor(
                                out=gated[:L, gsl], in0=pq[:L, :],
                                in1=gs[gq][:L, :], op=ALU.add)
                        # += self term s, broadcast over the 12 neighbors
                        sv = s_sb[:L, t * 2 * F:(t + 1) * 2 * F].rearrange(
                            "p (m c) -> p m c", m=1).to_broadcast([L, M, 2 * F])
                        gv = gated[:L, :].rearrange("p (m c) -> p m c", m=M)
                        nc.vector.tensor_tensor(out=gv, in0=gv, in1=sv, op=ALU.add)
                        for q5 in range(3):
                            gsl = slice(q5 * 512, (q5 + 1) * 512)
                            g2 = wp.tile([128, 512], BF16, tag="g2")
                            nc.scalar.square(g2[:L, :], gated[:L, gsl])
                            nc.tensor.matmul(
                                st_ps[:, :], ones_col_b[:L, :], gated[:L, gsl],
                                start=(t == 0 and q5 == 0),
                                stop=(t == NT - 1 and q5 == 2))
                            nc.tensor.matmul(
                                st2_ps[:, :], ones_col_b[:L, :], g2[:L, :],
                                start=(t == 0 and q5 == 0),
                                stop=(t == NT - 1 and q5 == 2))
                        nc.sync.dma_start(stash[t, :L, :], gated[:L, :])

                    str_sb = sp.tile([1, 1024], F32, tag="str")
                    nc.vector.tensor_copy(str_sb[:, :512], st_ps[:])
                    nc.vector.tensor_copy(str_sb[:, 512:], st2_ps[:])

                nc.sync.dma_start(cc_in[:], str_sb[:])
                nc.gpsimd.collective_compute(
                    "AllReduce", ALU.add, replica_groups=RG,
                    ins=[cc_in[:]], outs=[cc_out[:]])
                stg = sp.tile([1, 1024], F32, tag="str")
                nc.sync.dma_start(stg[:], cc_out[:])

                # fold the 4 j-copies, build affine rows
                mean = sp.tile([1, 2 * F], F32, tag="mean")
                nc.vector.reduce_sum(
                    mean[:], stg[:, :512].rearrange("p (j c) -> p c j", j=4),
                    axis=mybir.AxisListType.X)
                nc.vector.tensor_scalar_mul(mean[:], mean[:], 1.0 / (N * M))
                ex2 = sp.tile([1, 2 * F], F32, tag="ex2")
                nc.vector.reduce_sum(
                    ex2[:], stg[:, 512:].rearrange("p (j c) -> p c j", j=4),
                    axis=mybir.AxisListType.X)
                nc.vector.tensor_scalar_mul(ex2[:], ex2[:], 1.0 / (N * M))
                var = sp.tile([1, 2 * F], F32, tag="var")
                nc.vector.tensor_mul(var[:], mean[:], mean[:])
                nc.vector.tensor_sub(var[:], ex2[:], var[:])
                nc.vector.tensor_scalar_add(var[:], var[:], EPS)
                rstd = sp.tile([1, 2 * F], F32, tag="rstd")
                rsqrt_row(rstd[:], var[:], 2 * F, sp)
                g1 = sp.tile([1, 2 * F], F32, tag="g1r")
                nc.sync.dma_start(g1[:], bn1g[l])
                b1 = sp.tile([1, 2 * F], F32, tag="b1r")
                nc.sync.dma_start(b1[:], bn1b[l])
                sc_row = sp.tile([1, 2 * F], F32, tag="sc_row")
                nc.vector.tensor_mul(sc_row[:], rstd[:], g1[:])
                bi_row = sp.tile([1, 2 * F], F32, tag="bi_row")
                nc.vector.tensor_mul(bi_row[:], mean[:], sc_row[:])
                nc.vector.tensor_sub(bi_row[:], b1[:], bi_row[:])
                # negate the filt half: u = gated*sc+bi then holds [-filt | core],
                # so one softplus pass gives sp(-f) (sigmoid via exp) and sp(c)
                nc.vector.tensor_scalar_mul(sc_row[:, :F], sc_row[:, :F], -1.0)
                nc.vector.tensor_scalar_mul(bi_row[:, :F], bi_row[:, :F], -1.0)

                # broadcast to [128, 1536] bf16 via ones-matmul
                sc12 = sp.tile([1, M * 2 * F], BF16, tag="sc12")
                bi12 = sp.tile([1, M * 2 * F], BF16, tag="bi12")
                for m in range(M):
                    msl = slice(m * 2 * F, (m + 1) * 2 * F)
                    nc.vector.tensor_copy(sc12[:, msl], sc_row[:])
                    nc.vector.tensor_copy(bi12[:, msl], bi_row[:])
                ones_row_b = sp.tile([1, 128], BF16, tag="ones_row_b")
                nc.vector.memset(ones_row_b[:], 1.0)
                with tc.tile_pool(name=f"psR_{l}", bufs=2, space="PSUM") as psR:
                    for q3 in range(M * 2 * F // 512):
                        qsl = slice(q3 * 512, (q3 + 1) * 512)
                        rp = psR.tile([128, 512], F32, tag="rowps")
                        nc.tensor.matmul(rp[:], ones_row_b[:1, :], sc12[:, qsl],
                                         start=True, stop=True)
                        nc.vector.tensor_copy(scb12[:, qsl], rp[:])
                        rp2 = psR.tile([128, 512], F32, tag="rowps2")
                        nc.tensor.matmul(rp2[:], ones_row_b[:1, :], bi12[:, qsl],
                                         start=True, stop=True)
                        nc.vector.tensor_copy(bib12[:, qsl], rp2[:])

                # ---- pass B ----
                with tc.tile_pool(name=f"psB_{l}", bufs=1, space="PSUM") as psB:
                    s2_ps = psB.tile([1, 2 * F], F32, tag="s2ps")
                    for t, (o, L) in enumerate(TILES):
                        gt = wp.tile([128, M * 2 * F], BF16, tag="gatedB")
                        nc.sync.dma_start(gt[:L, :], stash[t, :L, :])
                        u = wp.tile([128, M * 2 * F], BF16, tag="u")
                        nc.vector.tensor_mul(u[:L, :], gt[:L, :], scb12[:L, :])
                        nc.vector.tensor_add(u[:L, :], u[:L, :], bib12[:L, :])
                        # A = softplus(u) for all 1536 cols: sp(-f) | sp(c)
                        A = wp.tile([128, M * 2 * F], BF16, tag="spA")
                        nc.scalar.activation(A[:L, :], u[:L, :], AF.Abs)
                        nc.scalar.activation(A[:L, :], A[:L, :], AF.Exp, scale=-1.0)
                        nc.scalar.activation(A[:L, :], A[:L, :], AF.Ln, bias=1.0)
                        nc.vector.scalar_tensor_tensor(
                            out=A[:L, :], in0=u[:L, :], scalar=0.0, in1=A[:L, :],
                            op0=ALU.max, op1=ALU.add)
                        Av = A[:L, :].rearrange("p (m c) -> p m c", m=M)
                        fi = wp.tile([128, M * F], BF16, tag="fi")
                        fv = fi[:L, :].rearrange("p (m c) -> p m c", m=M)
                        # sigmoid(f) = exp(-sp(-f))
                        nc.scalar.activation(fv, Av[:, :, :F], AF.Exp, scale=-1.0)
                        nc.vector.tensor_tensor(out=fv, in0=fv, in1=Av[:, :, F:],
                                                op=ALU.mult)
                        sm = sum_sb[:L, t * F:(t + 1) * F]
                        with nc.allow_low_precision(
                                reason="12-term sum in bf16; BN2 renormalizes"):
                            nc.vector.reduce_sum(
                                sm, fi[:L, :].rearrange("p (m c) -> p c m", m=M),
                                axis=mybir.AxisListType.X)
                        s2d = wp.tile([128, F], F32, tag="s2d")
                        nc.scalar.square(s2d[:L, :], sm)
                        nc.tensor.matmul(s2_ps[:, :F], ones_col_b[:L, :], sm,
                                         start=(t == 0), stop=False)
                        nc.tensor.matmul(s2_ps[:, F:], ones_col[:L, :], s2d[:L, :],
                                         start=(t == 0), stop=(t == NT - 1))
                    st2 = sp.tile([1, 2 * F], F32, tag="st2sb")
                    nc.vector.tensor_copy(st2[:], s2_ps[:])

                nc.sync.dma_start(cc2_in[:], st2[:])
                nc.gpsimd.collective_compute(
                    "AllReduce", ALU.add, replica_groups=RG,
                    ins=[cc2_in[:]], outs=[cc2_out[:]])
                stg2 = sp.tile([1, 2 * F], F32, tag="stg2")
                nc.sync.dma_start(stg2[:], cc2_out[:])
                mean2 = sp.tile([1, F], F32, tag="mean2")
                nc.vector.tensor_scalar_mul(mean2[:], stg2[:, :F], 1.0 / N)
                ex22 = sp.tile([1, F], F32, tag="ex22")
                nc.vector.tensor_scalar_mul(ex22[:], stg2[:, F:], 1.0 / N)
                var2 = sp.tile([1, F], F32, tag="var2")
                nc.vector.tensor_mul(var2[:], mean2[:], mean2[:])
                nc.vector.tensor_sub(var2[:], ex22[:], var2[:])
                nc.vector.tensor_scalar_add(var2[:], var2[:], EPS)
                rstd2 = sp.tile([1, F], F32, tag="rstd2")
                rsqrt_row(rstd2[:], var2[:], F, sp)
                g2r = sp.tile([1, F], F32, tag="g2rr")
                nc.sync.dma_start(g2r[:], bn2g[l])
                b2r = sp.tile([1, F], F32, tag="b2rr")
                nc.sync.dma_start(b2r[:], bn2b[l])
                sc2 = sp.tile([1, F], F32, tag="sc2")
                nc.vector.tensor_mul(sc2[:], rstd2[:], g2r[:])
                bi2 = sp.tile([1, F], F32, tag="bi2")
                nc.vector.tensor_mul(bi2[:], mean2[:], sc2[:])
                nc.vector.tensor_sub(bi2[:], b2r[:], bi2[:])
                sc2b = pp.tile([128, F], F32, tag="sc2b")
                bi2b = pp.tile([128, F], F32, tag="bi2b")
                with tc.tile_pool(name=f"psR2_{l}", bufs=1, space="PSUM") as psR2:
                    rp = psR2.tile([128, F], F32, tag="rowps")
                    nc.tensor.matmul(rp[:], ones_row[:1, :], sc2[:],
                                     start=True, stop=True)
                    nc.vector.tensor_copy(sc2b[:], rp[:])
                    rp2 = psR2.tile([128, F], F32, tag="rowps2")
                    nc.tensor.matmul(rp2[:], ones_row[:1, :], bi2[:],
                                     start=True, stop=True)
                    nc.vector.tensor_copy(bi2b[:], rp2[:])

                for t, (o, L) in enumerate(TILES):
                    sm = sum_sb[:L, t * F:(t + 1) * F]
                    asl = ain_sb[:L, t * F:(t + 1) * F]
                    v = wp.tile([128, F], F32, tag="vup")
                    nc.vector.tensor_mul(v[:L, :], sm, sc2b[:L, :])
                    nc.vector.tensor_add(v[:L, :], v[:L, :], bi2b[:L, :])
                    nc.vector.tensor_add(v[:L, :], v[:L, :], asl)
                    if not extra_res:
                        softplus_sep(aout_sb[:L, t * F:(t + 1) * F], v[:L, :])
                    else:
                        t1 = wp.tile([128, F], F32, tag="t1res")
                        softplus_sep(t1[:L, :], v[:L, :])
                        nc.vector.tensor_add(t1[:L, :], t1[:L, :], asl)
                        softplus_sep(aout_sb[:L, t * F:(t + 1) * F], t1[:L, :])

            conv_layer(0, a_sb, a1_sb, extra_res=False)
            conv_layer(1, a1_sb, a_sb, extra_res=True)  # a_sb <- final a2

            # ---- pooling ----
            iota_sb = sp.tile([128, CLOC], mybir.dt.int16, tag="iota")
            nc.gpsimd.iota(iota_sb[:], pattern=[[1, CLOC]], base=0,
                           channel_multiplier=0)

            cid_sb = sp.tile([128, NT], I32, tag="cid_sb")
            nc.sync.dma_start(cid_sb[:], cidp[:])
            with tc.tile_pool(name="psP", bufs=1, space="PSUM") as psP:
                cr_ps = psP.tile([F, CLOC], F32, tag="cr_ps")
                for t, (o, L) in enumerate(TILES):
                    cidf = wp.tile([128, 1], mybir.dt.int16, tag="cidf")
                    nc.vector.tensor_copy(cidf[:L, :], cid_sb[:L, t:t + 1])
                    A = wp.tile([128, CLOC], BF16, tag="Amat")
                    nc.vector.tensor_tensor(
                        out=A[:L, :], in0=iota_sb[:L, :],
                        in1=cidf[:L, :].to_broadcast([L, CLOC]),
                        op=ALU.is_equal)
                    a2b = wp.tile([128, F], BF16, tag="a2b")
                    nc.vector.tensor_copy(a2b[:L, :], a_sb[:L, t * F:(t + 1) * F])
                    nc.tensor.matmul(cr_ps[:], a2b[:L, :], A[:L, :],
                                     start=(t == 0), stop=(t == NT - 1))
                crl = wp.tile([F, CLOC], F32, tag="crl")
                nc.vector.tensor_copy(crl[:], cr_ps[:])
            nc.sync.dma_start(cr_in[:], crl[:])
            nc.gpsimd.collective_compute(
                "AllGather", ALU.bypass, replica_groups=RG,
                ins=[cr_in[:]], outs=[cr_out[:]])

            cg = pp.tile([F, N0P], F32, tag="cg")
            nc.vector.memset(cg[:], 0.0)
            for k in range(NC):
                w = wp.tile([F, CLOC], F32, tag="agw")
                nc.sync.dma_start(w[:], cr_out[k])
                nc.vector.tensor_add(cg[:, BASES[k]:BASES[k] + CLOC],
                                     cg[:, BASES[k]:BASES[k] + CLOC], w[:])
            NQH = (N0 + 511) // 512  # 4 head chunks cover all real crystals
            with tc.tile_pool(name="psH", bufs=2, space="PSUM") as psH:
                for q in range(NQH):
                    icr = wp.tile([1, 512], F32, tag="icr")
                    nc.sync.dma_start(icr[:], invc[:, q * 512:(q + 1) * 512])
                    icb = psH.tile([128, 512], F32, tag="icb")
                    nc.tensor.matmul(icb[:F, :], ones_row[:1, :F], icr[:],
                                     start=True, stop=True)
                    tmpc = wp.tile([F, CLOC], F32, tag="agw")
                    nc.vector.tensor_mul(tmpc[:],
                                         cg[:, q * 512:(q + 1) * 512], icb[:F, :])
                    softplus_sep(cg[:, q * 512:(q + 1) * 512], tmpc[:])

                wfc_sb = sp.tile([F + 1, H], F32, tag="wfc_sb")
                nc.sync.dma_start(wfc_sb[:], wfc[:])
                wout_sb = sp.tile([H, 1], F32, tag="wout_sb")
                nc.sync.dma_start(wout_sb[:], wout[:])
                bout_sb = sp.tile([1, 1], F32, tag="bout_sb")
                nc.sync.dma_start(bout_sb[:], bout[:])
                rhs = sp.tile([F + 1, 512], F32, tag="head_rhs")
                nc.vector.memset(rhs[F:F + 1, :], 1.0)
                for q in range(NQH):
                    nc.vector.tensor_copy(rhs[:F, :], cg[:, q * 512:(q + 1) * 512])
                    h_ps = psH.tile([128, 512], F32, tag="h_ps")
                    nc.tensor.matmul(h_ps[:H, :], wfc_sb[:], rhs[:],
                                     start=True, stop=True)
                    h_sb = wp.tile([H, 512], F32, tag="h_sb")
                    softplus_sep(h_sb[:], h_ps[:H, :])
                    o_ps = psH.tile([128, 512], F32, tag="o_ps")
                    nc.tensor.matmul(o_ps[:1, :], wout_sb[:], h_sb[:],
                                     start=True, stop=True)
                    oc = wp.tile([1, 512], F32, tag="ochunk")
                    nc.scalar.activation(oc[:], o_ps[:1, :], AF.Identity,
                                         bias=bout_sb[:, :1])
                    w = min(512, N0 - q * 512)
                    nc.sync.dma_start(out[:, q * 512:q * 512 + w], oc[:, :w])

    split_sync_waits(nc)
    return nc


_prog_cache = {}


def kernel(**inputs):
    atom_fea = np.asarray(inputs["atom_fea"], np.float32)
    nbr_fea = np.asarray(inputs["nbr_fea"], np.float32)
    nbr_fea_idx = np.asarray(inputs["nbr_fea_idx"]).astype(np.int32)
    crystal_id = np.asarray(inputs["crystal_id"]).astype(np.int32)

    W_emb = np.asarray(inputs["W_emb"], np.float32)
    b_emb = np.asarray(inputs["b_emb"], np.float32)
    wemb93 = np.vstack([W_emb, b_emb[None, :]]).astype(np.float32)

    def layer_w(Wn, bn):
        W = np.asarray(Wn, np.float32)
        b = np.asarray(bn, np.float32)
        wf = W[2 * F:]  # [NBR, 2F]
        wf3 = np.zeros((K3, GW), np.float32)
        for j in range(3):
            wf3[j * NBR:(j + 1) * NBR, j * 2 * F:(j + 1) * 2 * F] = wf
        # [s | p] combined: cols :2F = Wself rows + bias row; 2F: = Wnbr, 0 bias
        ws = np.zeros((F + 1, 4 * F), np.float32)
        ws[:F, :2 * F] = W[:F]
        ws[F, :2 * F] = b
        ws[:F, 2 * F:] = W[F:2 * F]
        return ws, wf3.astype(ml_dtypes.bfloat16)

    ws0, wf0 = layer_w(inputs["cW"], inputs["cb"])
    ws1, wf1 = layer_w(inputs["rW"], inputs["rb"])
    pack = lambda a, b: np.ascontiguousarray(np.stack([a, b]))
    wsp_np, wnbrf3_np = pack(ws0, ws1), pack(wf0, wf1)
    r1 = lambda k: np.asarray(inputs[k], np.float32)[None, :]
    bn1g_np, bn1b_np = pack(r1("cg1"), r1("rg1")), pack(r1("cbt1"), r1("rbt1"))
    bn2g_np, bn2b_np = pack(r1("cg2"), r1("rg2")), pack(r1("cbt2"), r1("rbt2"))
    wfc_np = np.vstack([np.asarray(inputs["W_fc"], np.float32),
                        np.asarray(inputs["b_fc"], np.float32)[None, :]])
    wout_np = np.asarray(inputs["W_out"], np.float32)
    bout_np = np.asarray(inputs["b_out"], np.float32).reshape(1, 1)

    cnt = np.bincount(crystal_id, minlength=N0).astype(np.float32)
    inv = (1.0 / np.maximum(cnt, 1.0)).astype(np.float32)
    invc_np = np.zeros((1, N0P), np.float32)
    invc_np[0, :N0] = inv

    nbrT_b = nbr_fea.astype(ml_dtypes.bfloat16)
    atomT_full = np.vstack([atom_fea.T, np.ones((1, N), np.float32)])

    def pack_nbr3(nb, lo):
        # [NT, 123, 512]: partitions j*41+k (j=0..2), cols g*128+p hold
        # nb[lo + t*128 + p, g*3+j, k]
        blk = nb[lo:lo + S]  # [S, 12, 41]
        padded = np.zeros((NT * 128, M, NBR), ml_dtypes.bfloat16)
        padded[:S] = blk
        x = padded.reshape(NT, 128, G4, 3, NBR)
        x = np.transpose(x, (0, 3, 4, 2, 1))  # [NT, 3, 41, 4, 128]
        return np.ascontiguousarray(x.reshape(NT, K3, 512))

    def pack_rows(arr, lo, ncol):
        # [S(+pad), ncol] -> [128, NT*ncol]: out[p, t*ncol+j] = arr[lo+t*128+p, j]
        pad = np.zeros((NT * 128, ncol), np.int32)
        pad[:S] = arr[lo:lo + S].reshape(S, ncol)
        return np.ascontiguousarray(
            pad.reshape(NT, 128, ncol).transpose(1, 0, 2).reshape(128, NT * ncol))

    in_maps = []
    for c in range(NC):
        lo, hi = c * S, (c + 1) * S
        cidl_np = (crystal_id[lo:hi] - BASES[c]).astype(np.int32)
        assert cidl_np.min() >= 0 and cidl_np.max() < CLOC, (
            f"core {c}: crystal window [{cidl_np.min()},{cidl_np.max()}]")
        in_maps.append({
            "atomT": np.ascontiguousarray(atomT_full[:, lo:hi]),
            "nbrT": pack_nbr3(nbrT_b, lo),
            "idxp": pack_rows(nbr_fea_idx, lo, M),
            "cidp": pack_rows(crystal_id.astype(np.int32) - BASES[c], lo, 1),
            "invc": invc_np,
            "wemb": wemb93, "wsp": wsp_np,
            "wnbrf3": wnbrf3_np, "bn1g": bn1g_np, "bn1b": bn1b_np,
            "bn2g": bn2g_np, "bn2b": bn2b_np,
            "wfc": wfc_np, "wout": wout_np, "bout": bout_np,
        })

    if "prog" not in _prog_cache:
        _prog_cache["prog"] = build_program()
    nc = _prog_cache["prog"]

    trace = bool(int(os.environ.get("KERNEL_TRACE", "0")))
    res = run_bass_kernel_spmd(nc, in_maps, core_ids=list(range(NC)), trace=trace)
    if trace:
        kernel.last_exec_ns = res.exec_time_ns
        kernel.last_trace = (res.instructions_and_trace or (None, None))[1]
    return res.results[0]["out"].reshape(N0, 1).astype(np.float32)

